# revision 16
# baseline (speedup 1.0000x reference)
"""Trainium2 Bass kernel for CongestionAwareMP (GNN message passing).

Math (reference):
    msg_in = [x[src], x[dst], edge_attr, cong[src]]          # [E, 289]
    h      = relu(msg_in @ mW1 + mb1)                        # [E, 256]
    msgs   = h @ mW2 + mb2                                   # [E, 128]
    agg    = segment_sum(msgs, dst, N)                       # [N, 128]
    h2     = relu([x, agg] @ uW1 + ub1)                      # [N, 256]
    out    = h2 @ uW2 + ub2                                  # [N, 128]

Kernel decomposition (linear-algebra identities, exact up to bf16 rounding):
  * mW1 splits by input block:  h = relu(A[src] + B[dst] + ea @ W1e)
      A = x @ mW1[:128] + cong * mW1[288] + mb1   (per-node table)
      B = x @ mW1[128:256]                        (per-node table)
  * segment_sum commutes with the linear mW2 map:
      agg = segment_sum(h) @ mW2 + deg * mb2
  * mW2 folds into the update MLP (host-side weight product):
      h2 = relu(x @ uW1a + hsum @ V + deg * c + ub1),  V = mW2 @ uW1b

Distribution: edges sharded by dst node range (node-parallel).  Each core
computes the A-table rows for its own node shard, then the tables are
exchanged with one on-device AllGather (6.4 MB/core).  A[src] rows are
fetched per edge with a gpsimd DMA gather (the per-index descriptor rate
of ~8 ns/row is the kernel's floor); B[dst] is selected on the PE array
via an exact one-hot built from a partition-broadcast of the dst ids —
no second gather.  Aggregation uses the same one-hot transposed.
"""

import math
import os
import sys

sys.path.insert(0, "/opt/trn_rl_repo")

import ml_dtypes
import numpy as np

BF16 = ml_dtypes.bfloat16

P = 128          # partitions
WIN = 256        # dst-window (nodes) for aggregation
NG = 4           # src-range groups (int16 gather index limit)
D = 128          # node feature dim
ED = 32          # edge feature dim
HID = 256        # hidden dim

JAX_CACHE_DIR = "/tmp/bass_jax_cache"


def _cfg(n_nodes, n_edges, n_cores):
    Sr = int(math.ceil(n_nodes / (n_cores * WIN))) * WIN  # nodes per core
    Npad = Sr * n_cores              # global padded node space
    GRP = Npad // NG                 # == 2 * Sr when NG == n_cores // 2
    return dict(N=n_nodes, E=n_edges, NC=n_cores, Sr=Sr, Npad=Npad,
                Wr=Sr // WIN, GRP=GRP)


def _host_prep(x, edge_index, edge_attr, congestion,
               mW1, mb1, mW2, mb2, uW1, ub1, uW2, ub2, n_cores):
    cfg = _cfg(x.shape[0], edge_index.shape[1], n_cores)
    N, E, NC, Sr, Npad, Wr, GRP = (cfg[k] for k in
                                   ("N", "E", "NC", "Sr", "Npad", "Wr", "GRP"))

    x = np.asarray(x, np.float32)
    ea = np.asarray(edge_attr, np.float32)
    cong = np.asarray(congestion, np.float32)
    src = np.asarray(edge_index[0]).astype(np.int32, copy=False)
    dst = np.asarray(edge_index[1]).astype(np.int32, copy=False)
    mW1 = np.asarray(mW1, np.float32); mb1 = np.asarray(mb1, np.float32)
    mW2 = np.asarray(mW2, np.float32); mb2 = np.asarray(mb2, np.float32)
    uW1 = np.asarray(uW1, np.float32); ub1 = np.asarray(ub1, np.float32)
    uW2 = np.asarray(uW2, np.float32); ub2 = np.asarray(ub2, np.float32)

    # ---- global ordering by (dst-window, src-group) ----
    # single combined sort: key * E + edge_id  (stable by construction)
    key = (dst // WIN) * NG + src // GRP
    comb = key.astype(np.int64) * E + np.arange(E, dtype=np.int64)
    comb.sort()
    keys = (comb // E).astype(np.int32)
    order = (comb % E).astype(np.int32)

    nbuck = NC * Wr * NG
    gcnt = np.bincount(keys, minlength=nbuck)
    T_G = max(1, int(math.ceil(gcnt.max() / P)))
    T_W = NG * T_G
    Tt = Wr * T_W                    # tiles per core (real windows only)
    cfg.update(T_G=T_G, T_W=T_W, Tt=Tt)

    kstart = np.zeros(nbuck, np.int64)
    np.cumsum(gcnt[:-1], out=kstart[1:])
    rank = np.arange(E, dtype=np.int64) - kstart[keys]
    assert rank.max(initial=0) < T_G * P
    core = keys // (Wr * NG)
    klocal = keys - core * (Wr * NG)
    gslot = core * (Tt * P) + klocal * (T_G * P) + rank
    nslots_g = NC * Tt * P

    src_s = src[order]
    dst_s = dst[order]
    srcl_g = np.zeros(nslots_g, np.int16)
    srcl_g[gslot] = (src_s % GRP).astype(np.int16)
    dstl16 = (dst_s % WIN).astype(np.int16)
    dstf_g = np.full(nslots_g, -1.0, np.float32)
    dstf_g[gslot] = dstl16.astype(np.float32)      # 0..255 exact
    dstr_g = np.full(nslots_g, -1.0, BF16)
    dstr_g[gslot] = dstl16.astype(BF16)            # 0..255 exact in bf16
    eat_g = np.zeros((ED, nslots_g), BF16)
    eat_g[:, gslot] = ea.astype(BF16)[order].T

    deg_full = np.bincount(dst, minlength=Npad).astype(BF16)

    # ---- shared (replicated) small tensors ----
    w1ab = np.concatenate([mW1[0:D], mW1[D:2 * D]], axis=1).astype(BF16)  # [128, 512]
    w1cb1 = np.stack([mW1[2 * D + ED], mb1]).astype(BF16)                 # [2, 256]
    w1e = mW1[2 * D:2 * D + ED].astype(BF16)                              # [32, 256]
    uW1a = uW1[0:D].astype(BF16)
    uW1b = uW1[D:2 * D]
    V = (mW2 @ uW1b).astype(BF16)
    c2 = (mb2 @ uW1b)[None, :].astype(BF16)
    iota = np.broadcast_to(np.arange(WIN, dtype=np.float32), (P, WIN)).astype(BF16)
    iotacol = np.stack([np.arange(P, dtype=np.float32),
                        np.arange(P, dtype=np.float32) + P], axis=1)
    shared = dict(
        w1ab=w1ab, w1cb1=w1cb1, w1e=w1e, uw1a=uW1a,
        v0=V[0:P].copy(), v1=V[P:2 * P].copy(), c2=c2,
        uw2a=uW2[0:P].astype(BF16), uw2b=uW2[P:2 * P].astype(BF16),
        ub2=ub2[None, :].astype(BF16), ones1=np.ones((1, D), BF16),
        ub1c=np.stack([ub1[0:P], ub1[P:2 * P]], axis=1).astype(np.float32),
        iota=np.ascontiguousarray(iota),
        iotacol=np.ascontiguousarray(iotacol),
    )

    # x^T in bf16 once, then per-core column slices
    xT = np.ascontiguousarray(x.astype(BF16).T)   # [128, N]

    in_maps = []
    for c in range(NC):
        r0, r1 = c * Sr, min((c + 1) * Sr, N)
        xsh = np.zeros((P, Sr), BF16)
        xsh[:, :r1 - r0] = xT[:, r0:r1]
        cosh = np.ones((2, Sr), BF16)
        cosh[0] = 0.0
        cosh[0, :r1 - r0] = cong[r0:r1].astype(BF16)

        sl = slice(c * Tt * P, (c + 1) * Tt * P)
        m = dict(shared)
        m["xsh"] = xsh
        m["cosh"] = cosh
        Wr_, T_W_ = Wr, Tt // Wr
        m["srclc"] = np.ascontiguousarray(
            srcl_g[sl].reshape(Wr_, T_W_ * 8, 16).transpose(0, 2, 1))     # [Wr, 16, T_W*8]
        m["dstfc"] = np.ascontiguousarray(
            dstf_g[sl].reshape(Wr_, T_W_, P).transpose(0, 2, 1))          # [Wr, 128, T_W] f32
        m["dstr"] = dstr_g[None, sl]                                      # [1, Tt*128]
        m["eat"] = eat_g[:, sl]                                           # [32, Tt*128]
        m["deg"] = deg_full[None, c * Sr:(c + 1) * Sr]                    # [1, Sr]
        in_maps.append(m)

    return cfg, in_maps


def input_specs(cfg):
    Sr, Tt = cfg["Sr"], cfg["Tt"]
    return {
        "xsh": ((P, Sr), BF16), "cosh": ((2, Sr), BF16),
        "w1ab": ((P, 2 * HID), BF16), "w1cb1": ((2, HID), BF16),
        "w1e": ((ED, HID), BF16), "uw1a": ((P, HID), BF16),
        "v0": ((P, HID), BF16), "v1": ((P, HID), BF16),
        "c2": ((1, HID), BF16), "uw2a": ((P, D), BF16), "uw2b": ((P, D), BF16),
        "ub2": ((1, D), BF16), "ones1": ((1, D), BF16),
        "ub1c": ((P, 2), np.float32), "iota": ((P, WIN), BF16),
        "iotacol": ((P, 2), np.float32),
        "srclc": ((Sr // WIN, 16, (Tt // (Sr // WIN)) * 8), np.int16),
        "dstfc": ((Sr // WIN, P, Tt // (Sr // WIN)), np.float32),
        "dstr": ((1, Tt * P), BF16),
        "eat": ((ED, Tt * P), BF16),
        "deg": ((1, Sr), BF16),
    }


def build(tc, ins, outs, cfg):
    """Emit the Tile kernel.  ins/outs: dict name -> bass.AP (DRAM)."""
    from contextlib import ExitStack

    import concourse.mybir as mybir
    from concourse.tile_rust import add_dep_helper

    nc = tc.nc
    dt = mybir.dt
    AF = mybir.ActivationFunctionType
    ALU = mybir.AluOpType
    NC, Sr, Npad, Wr, T_W, T_G, GRP, Tt = (
        cfg[k] for k in ("NC", "Sr", "Npad", "Wr", "T_W", "T_G", "GRP", "Tt"))

    with ExitStack() as ctx:
        wp = ctx.enter_context(tc.tile_pool(name="wts", bufs=1))

        def load_w(name, shape, dty=dt.bfloat16):
            t = wp.tile(list(shape), dty, name=f"w_{name}")
            nc.sync.dma_start(out=t[:], in_=ins[name][:])
            return t

        w1ab = load_w("w1ab", (P, 2 * HID))
        w1cb1 = load_w("w1cb1", (2, HID))
        w1e = load_w("w1e", (ED, HID))
        uw1a = load_w("uw1a", (P, HID))
        v0 = load_w("v0", (P, HID))
        v1 = load_w("v1", (P, HID))
        c2 = load_w("c2", (1, HID))
        uw2a = load_w("uw2a", (P, D))
        uw2b = load_w("uw2b", (P, D))
        ub2 = load_w("ub2", (1, D))
        ones1 = load_w("ones1", (1, D))
        ub1c = load_w("ub1c", (P, 2), dt.float32)
        iota = load_w("iota", (P, WIN))
        iotacol = load_w("iotacol", (P, 2), dt.float32)
        deg = load_w("deg", (1, Sr))

        # A table is exchanged via AllGather; B stays core-local.
        agin = nc.dram_tensor("agin", [Sr, HID], dt.bfloat16, kind="Internal")
        agout = nc.dram_tensor("agout", [NC, Sr, HID], dt.bfloat16,
                               kind="Internal", addr_space="Shared")
        Bt = nc.dram_tensor("Bt", [Sr, HID], dt.bfloat16, kind="Internal")
        BtA = Bt.ap()

        # ---------- broadcast gather indices to 8 gpsimd cores ----------
        # [Wr, P, T_W*8]: each window's replicated index block is contiguous,
        # so the per-window SBUF load is one big descriptor, not 128 strided.
        sidx = nc.dram_tensor("sidx", [Wr, P, T_W * 8], dt.int16, kind="Internal")
        idx_writes = []
        for k in range(8):
            idx_writes.append(nc.sync.dma_start(
                out=sidx.ap()[:, 16 * k:16 * (k + 1), :], in_=ins["srclc"][:]))
        jidx = nc.engines[mybir.EngineType.SP].nop(nofuse=True, hint="idx_ready")
        for wi in idx_writes:
            add_dep_helper(jidx.ins, wi.ins, reason="idx bcast write")

        # ---------- per-node tables: A (own shard) + B, then AllGather A ----
        a_writes = []
        b_writes = {}          # chunk index -> join instruction
        with tc.tile_pool(name="pp", bufs=3) as pp, \
             tc.tile_pool(name="pps", bufs=3, space="PSUM") as pps:
            chunks = [(o, 512) for o in range(0, Sr - 511, 512)]
            if Sr % 512:
                chunks.append((Sr - Sr % 512, Sr % 512))

            for ci, (off, csz) in enumerate(chunks):
                xs = pp.tile([P, csz], dt.bfloat16, name="xs4")
                nc.sync.dma_start(out=xs[:], in_=ins["xsh"][:, off:off + csz])
                co4 = pp.tile([2, csz], dt.bfloat16, name="co4")
                nc.sync.dma_start(out=co4[:], in_=ins["cosh"][:, off:off + csz])
                cwr = []
                for k in range(csz // P):
                    ps = pps.tile([P, 2 * HID], dt.float32, name="ppsb")
                    nc.tensor.matmul(ps[:, 0:HID], lhsT=xs[:, k * P:(k + 1) * P],
                                     rhs=w1ab[:, 0:HID], start=True, stop=False)
                    nc.tensor.matmul(ps[:, 0:HID], lhsT=co4[:, k * P:(k + 1) * P],
                                     rhs=w1cb1[:], start=False, stop=True)
                    nc.tensor.matmul(ps[:, HID:2 * HID], lhsT=xs[:, k * P:(k + 1) * P],
                                     rhs=w1ab[:, HID:2 * HID], start=True, stop=True)
                    cb = pp.tile([P, 2 * HID], dt.bfloat16, name="cbuf")
                    nc.scalar.copy(out=cb[:, 0:HID], in_=ps[:, 0:HID])
                    nc.vector.tensor_copy(out=cb[:, HID:2 * HID], in_=ps[:, HID:2 * HID])
                    wa = nc.sync.dma_start(
                        out=agin.ap()[off + k * P:off + (k + 1) * P, :],
                        in_=cb[:, 0:HID])
                    a_writes.append(wa)
                    wb = nc.sync.dma_start(
                        out=BtA[off + k * P:off + (k + 1) * P, :],
                        in_=cb[:, HID:2 * HID])
                    cwr.append(wb)
                jb = nc.engines[mybir.EngineType.SP].nop(nofuse=True,
                                                         hint=f"B_{ci}")
                for wi in cwr:
                    add_dep_helper(jb.ins, wi.ins, reason="B chunk write")
                b_writes[ci] = jb

        cc = nc.gpsimd.collective_compute(
            "AllGather", ALU.bypass, [list(range(NC))],
            ins=[agin.ap()[:, :]], outs=[agout.ap()[:, :, :]])
        for wa in a_writes:
            add_dep_helper(cc.ins, wa.ins, reason="allgather after A build")

        def bjoin(w):
            # B rows for window w live in column chunk (w*WIN)//512
            return b_writes[min((w * WIN) // 512, len(b_writes) - 1)]

        # ---------- edge phase + fused update ----------
        with tc.tile_pool(name="ew", bufs=3) as ew, \
             tc.tile_pool(name="esg", bufs=3) as esg, \
             tc.tile_pool(name="esb", bufs=3) as esb, \
             tc.tile_pool(name="es", bufs=2) as es, \
             tc.tile_pool(name="eo", bufs=6) as eo, \
             tc.tile_pool(name="eps", bufs=2, space="PSUM") as eps, \
             tc.tile_pool(name="aps", bufs=2, space="PSUM") as aps, \
             tc.tile_pool(name="up", bufs=2) as up, \
             tc.tile_pool(name="ups", bufs=1, space="PSUM") as ups, \
             tc.tile_pool(name="ops", bufs=1, space="PSUM") as ops:
            pending_update = [None]

            def flush_update():
                if pending_update[0] is not None:
                    pending_update[0]()
                    pending_update[0] = None

            for w in range(Wr):
                t0 = w * T_W
                srcl_w = ew.tile([P, T_W * 8], dt.int16, name="srcl_w")
                ld = nc.sync.dma_start(out=srcl_w[:], in_=sidx.ap()[w, :, :])
                add_dep_helper(ld.ins, jidx.ins, reason="idx after bcast")
                dstf_w = ew.tile([P, T_W], dt.float32, name="dstf_w")
                nc.sync.dma_start(out=dstf_w[:], in_=ins["dstfc"][w, :, :])
                xtw = ew.tile([P, WIN], dt.bfloat16, name="xtw")
                nc.sync.dma_start(out=xtw[:], in_=ins["xsh"][:, w * WIN:(w + 1) * WIN])
                dstbc = esb.tile([P, T_W * P], dt.bfloat16, name="dstbc")
                nq = T_W * P // 4
                for q in range(4):
                    nc.sync.dma_start(
                        out=dstbc[:, q * nq:(q + 1) * nq],
                        in_=ins["dstr"][0:1, t0 * P + q * nq:
                                        t0 * P + (q + 1) * nq].partition_broadcast(P))
                eat_w = es.tile([ED, T_W * P], dt.bfloat16, name="eat_w")
                nc.sync.dma_start(out=eat_w[:], in_=ins["eat"][:, t0 * P:(t0 + T_W) * P])
                bw = ew.tile([P, 2 * HID], dt.bfloat16, name="bw")
                for g in range(2):
                    lb = nc.sync.dma_start(
                        out=bw[:, g * HID:(g + 1) * HID],
                        in_=BtA[w * WIN + g * P:w * WIN + (g + 1) * P, :])
                    add_dep_helper(lb.ins, bjoin(w).ins, reason="Bw after B build")

                GA = esg.tile([P, T_W * HID], dt.bfloat16, name="GA")
                for g in range(NG):
                    ga = nc.gpsimd.dma_gather(
                        GA[:, g * T_G * HID:(g + 1) * T_G * HID].rearrange(
                            "p (t e) -> p t e", e=HID),
                        agout.ap()[2 * g:2 * g + 2, :, :].rearrange(
                            "c s h -> (c s) h"),
                        srcl_w[:, g * T_G * 8:(g + 1) * T_G * 8],
                        T_G * P, T_G * P, HID,
                        single_packet=False)
                    add_dep_helper(ga.ins, cc.ins, reason="gatherA after AG")

                # hoisted one-hot builds for the whole window (no chain deps).
                # Split across engines: vector via is_equal, scalar via the
                # exact integer identity  onehot = relu(1 - |iota - dst|).
                # fp8 output: 0/1 are exact, and it feeds the DoubleRow
                # aggregation matmul directly.
                ohb = es.tile([P, T_W * WIN], dt.float8e4, name="ohb")
                for t in range(T_W):
                    sl = ohb[:, t * WIN:(t + 1) * WIN]
                    if t % 2 == 0:
                        nc.vector.tensor_scalar(sl, iota[:],
                                                dstf_w[:, t:t + 1], None,
                                                op0=ALU.is_equal)
                    else:
                        nc.scalar.activation(sl, iota[:], AF.Abs,
                                             bias=dstf_w[:, t:t + 1], scale=-1.0)
                        nc.scalar.activation(sl, sl, AF.Relu, bias=1.0, scale=-1.0)
                hbuf = es.tile([P, T_W * HID], dt.float8e4, name="hbuf")

                hs0 = aps.tile([P, WIN], dt.float32, name="hsum0")
                hs1 = aps.tile([P, WIN], dt.float32, name="hsum1")

                # software-pipelined: chunk i's select matmuls are issued
                # before chunk i-1's aggregation matmuls so the in-order PE
                # queue never waits on the vector/scalar add+relu of i-1.
                NCH = T_W // 2
                ps2s = [None] * NCH
                ohTs = [None] * NCH
                for i in range(NCH + 2):
                    if i < NCH:
                        if i % 2 == 0:
                            # one-hot^T batched over 4 tiles (two chunks)
                            nb = min(4 * P, (T_W - 2 * i) * P)
                            ohT0 = eo.tile([P, 4 * P], dt.bfloat16, name="ohT0")
                            nc.vector.tensor_scalar(
                                ohT0[:, 0:nb],
                                dstbc[:, i * 2 * P:i * 2 * P + nb],
                                iotacol[:, 0:1], None, op0=ALU.is_equal)
                            ohT1 = eo.tile([P, 4 * P], dt.bfloat16, name="ohT1")
                            nc.vector.tensor_scalar(
                                ohT1[:, 0:nb],
                                dstbc[:, i * 2 * P:i * 2 * P + nb],
                                iotacol[:, 1:2], None, op0=ALU.is_equal)
                            ohTs[i] = (ohT0, ohT1, 0)
                            if i + 1 < NCH:
                                ohTs[i + 1] = (ohT0, ohT1, 2 * P)
                        ohT0, ohT1, obase = ohTs[i]
                        ps2 = eps.tile([P, 2 * HID], dt.float32, name="ps2")
                        ps2s[i] = ps2
                        for j in range(2):
                            t = 2 * i + j
                            slp = ps2[:, j * HID:(j + 1) * HID]
                            nc.tensor.matmul(slp, lhsT=eat_w[:, t * P:(t + 1) * P],
                                             rhs=w1e[:], start=True, stop=False)
                            nc.tensor.matmul(slp,
                                             lhsT=ohT0[:, obase + j * P:obase + (j + 1) * P],
                                             rhs=bw[:, 0:HID], start=False, stop=False)
                            nc.tensor.matmul(slp,
                                             lhsT=ohT1[:, obase + j * P:obase + (j + 1) * P],
                                             rhs=bw[:, HID:2 * HID], start=False,
                                             stop=True)
                    if 1 <= i <= NCH:
                        c = i - 1
                        GA2 = GA[:, c * 2 * HID:(c + 1) * 2 * HID]
                        nc.vector.tensor_tensor(out=GA2, in0=GA2, in1=ps2s[c][:],
                                                op=ALU.add)
                        hb2 = hbuf[:, c * 2 * HID:(c + 1) * 2 * HID]
                        nc.scalar.activation(hb2, GA2, AF.Relu)
                    if i >= 2:
                        c = i - 2
                        # DoubleRow aggregation: both tiles of the chunk in
                        # one matmul per hid-half (K = 2x128 edge slots).
                        hb2 = hbuf[:, c * 2 * HID:(c + 1) * 2 * HID]
                        oh_r = ohb[:, 2 * c * WIN:(2 * c + 2) * WIN].rearrange(
                            "p (k n) -> p k n", k=2)
                        hb_r = hb2.rearrange("p (k n) -> p k n", k=2)
                        first, last = (c == 0), (c == NCH - 1)
                        nc.tensor.matmul(hs0[:], lhsT=hb_r[:, :, 0:P], rhs=oh_r,
                                         start=first, stop=last,
                                         perf_mode=mybir.MatmulPerfMode.DoubleRow)
                        nc.tensor.matmul(hs1[:], lhsT=hb_r[:, :, P:2 * P], rhs=oh_r,
                                         start=first, stop=last,
                                         perf_mode=mybir.MatmulPerfMode.DoubleRow)

                # ----- update MLP for this window's nodes -----
                # Deferred into the next window's chunk loop so the PE queue
                # is never drained at a window boundary.
                def make_update(w, hs0, hs1, xtw):
                    def emit():
                        hsb = up.tile([P, 2 * HID], dt.bfloat16, name="hsb")
                        nc.vector.tensor_copy(out=hsb[:, 0:HID], in_=hs0[:])
                        nc.vector.tensor_copy(out=hsb[:, HID:2 * HID], in_=hs1[:])
                        psu = ups.tile([P, 2 * HID], dt.float32, name="psu")
                        for j in range(2):
                            slu = psu[:, j * HID:(j + 1) * HID]
                            nc.tensor.matmul(slu, lhsT=uw1a[:, j * P:(j + 1) * P],
                                             rhs=xtw[:], start=True, stop=False)
                            nc.tensor.matmul(slu, lhsT=v0[:, j * P:(j + 1) * P],
                                             rhs=hsb[:, 0:HID], start=False, stop=False)
                            nc.tensor.matmul(slu, lhsT=v1[:, j * P:(j + 1) * P],
                                             rhs=hsb[:, HID:2 * HID], start=False,
                                             stop=False)
                            nc.tensor.matmul(slu, lhsT=c2[:, j * P:(j + 1) * P],
                                             rhs=deg[:, w * WIN:(w + 1) * WIN],
                                             start=False, stop=True)
                        h2 = up.tile([P, 2 * HID], dt.bfloat16, name="h2")
                        for j in range(2):
                            nc.scalar.activation(h2[:, j * HID:(j + 1) * HID],
                                                 psu[:, j * HID:(j + 1) * HID],
                                                 AF.Relu, bias=ub1c[:, j:j + 1])
                        pso = ops.tile([P, WIN], dt.float32, name="pso")
                        for sx in range(2):
                            slo = pso[:, sx * D:(sx + 1) * D]
                            nc.tensor.matmul(slo, lhsT=h2[:, sx * P:sx * P + P],
                                             rhs=uw2a[:], start=True, stop=False)
                            nc.tensor.matmul(slo,
                                             lhsT=h2[:, HID + sx * P:HID + sx * P + P],
                                             rhs=uw2b[:], start=False, stop=False)
                            nc.tensor.matmul(slo, lhsT=ones1[:], rhs=ub2[:],
                                             start=False, stop=True)
                            osb = up.tile([P, D], dt.bfloat16, name="osb")
                            nc.scalar.copy(out=osb[:], in_=slo)
                            nc.sync.dma_start(
                                out=outs["xnew"][w * WIN + sx * P:
                                                 w * WIN + (sx + 1) * P, :],
                                in_=osb[:])
                    return emit

                make_update(w, hs0, hs1, xtw)()
            flush_update()


def _install_ntff_hook():
    """Make ``antenv.axon_hooks`` importable so bass_utils' axon trace path
    finds the NTFF profile hook (the boot skips registration when the
    image's antenv package lacks the module).  Returns True when the hook
    is available, False otherwise (tracing then degrades gracefully)."""
    import types

    try:
        from antenv.axon_hooks import get_axon_ntff_profile_hook
        return get_axon_ntff_profile_hook() is not None
    except ImportError:
        pass
    try:
        from trn_agent_boot.trn_boot import _ntff_profile_via_ctypes
        hook = _ntff_profile_via_ctypes("/opt/axon/libaxon_pjrt.so")
    except Exception:
        return False
    if hook is None:
        return False
    mod = types.ModuleType("antenv.axon_hooks")
    mod._hook = hook
    mod.get_axon_ntff_profile_hook = lambda: mod._hook
    mod.set_axon_ntff_profile_hook = lambda h: setattr(mod, "_hook", h)
    sys.modules["antenv.axon_hooks"] = mod
    return True


_RUN_CACHE = {}


def _run_pjrt(nc, in_maps, n_cores):
    """Mirror of bass2jax.run_bass_via_pjrt with two transfer
    optimizations for the ~70 MB/s axon host link: the donated
    ExternalOutput zero buffers are created on-device (saves shipping
    26 MB of zeros), and host->device uploads run on a small thread
    pool (the tunnel gains ~10% with concurrent streams)."""
    from concurrent.futures import ThreadPoolExecutor

    import jax
    import jax.numpy as jnp
    from jax.sharding import Mesh, NamedSharding, PartitionSpec
    from jax.experimental.shard_map import shard_map

    import concourse.mybir as mybir
    from concourse.bass2jax import (_bass_exec_p, install_neuronx_cc_hook,
                                    partition_id_tensor)

    key = id(nc)
    if key not in _RUN_CACHE:
        install_neuronx_cc_hook()
        partition_name = (nc.partition_id_tensor.name
                          if nc.partition_id_tensor else None)
        in_names, out_names, out_avals = [], [], []
        for alloc in nc.m.functions[0].allocations:
            if not isinstance(alloc, mybir.MemoryLocationSet):
                continue
            name = alloc.memorylocations[0].name
            if alloc.kind == "ExternalInput":
                if name != partition_name:
                    in_names.append(name)
            elif alloc.kind == "ExternalOutput":
                out_names.append(name)
                out_avals.append(jax.core.ShapedArray(
                    tuple(alloc.tensor_shape), mybir.dt.np(alloc.dtype)))
        n_params = len(in_names)
        n_outs = len(out_avals)
        in_names = in_names + out_names
        if partition_name is not None:
            in_names.append(partition_name)

        def _body(*args):
            operands = list(args)
            if partition_name is not None:
                operands.append(partition_id_tensor())
            return tuple(_bass_exec_p.bind(
                *operands, out_avals=tuple(out_avals),
                in_names=tuple(in_names), out_names=tuple(out_names),
                lowering_input_output_aliases=(),
                sim_require_finite=True, sim_require_nnan=True, nc=nc))

        devices = jax.devices()[:n_cores]
        assert len(devices) == n_cores
        mesh = Mesh(np.asarray(devices), ("core",))
        spec = PartitionSpec("core")
        sh = NamedSharding(mesh, spec)
        sharded = jax.jit(
            shard_map(_body, mesh=mesh, in_specs=(spec,) * (n_params + n_outs),
                      out_specs=(spec,) * n_outs),
            donate_argnums=tuple(range(n_params, n_params + n_outs)),
            keep_unused=True)
        zeros_fn = jax.jit(
            lambda: tuple(jnp.zeros((n_cores * a.shape[0], *a.shape[1:]),
                                    a.dtype) for a in out_avals),
            out_shardings=(sh,) * n_outs)
        _RUN_CACHE[key] = (in_names, out_names, out_avals, n_params, n_outs,
                           sh, sharded, zeros_fn)
    (in_names, out_names, out_avals, n_params, n_outs,
     sh, sharded, zeros_fn) = _RUN_CACHE[key]

    concat_in = [
        np.concatenate([np.asarray(m[name]) for m in in_maps], axis=0)
        for name in in_names[:n_params]]
    dev_zeros = zeros_fn()

    # byte-balanced upload groups over a few concurrent tunnel streams
    order = sorted(range(n_params), key=lambda i: -concat_in[i].nbytes)
    groups = [[] for _ in range(4)]
    sizes = [0] * 4
    for i in order:
        g = sizes.index(min(sizes))
        groups[g].append(i)
        sizes[g] += concat_in[i].nbytes
    dev_in = [None] * n_params

    def _upload(group):
        for i in group:
            dev_in[i] = jax.device_put(concat_in[i], sh)
        for i in group:
            dev_in[i].block_until_ready()

    with ThreadPoolExecutor(4) as ex:
        list(ex.map(_upload, groups))

    out_arrs = sharded(*dev_in, *dev_zeros)
    for o in out_arrs:
        o.block_until_ready()
    results = [
        {name: np.asarray(out_arrs[i]).reshape(n_cores, *out_avals[i].shape)[c]
         for i, name in enumerate(out_names)}
        for c in range(n_cores)]

    class _Res:
        pass

    res = _Res()
    res.results = results
    res.exec_time_ns = None
    return res


_CACHE = {}


def _compiled(cfg):
    key = ("v2", cfg["N"], cfg["E"], cfg["NC"], cfg["T_W"])
    if key in _CACHE:
        return _CACHE[key]
    import concourse.mybir as mybir
    import concourse.tile as tile
    from concourse import bacc

    nc = bacc.Bacc("TRN2", target_bir_lowering=False, debug=False,
                   enable_asserts=False, num_devices=cfg["NC"])
    ins = {}
    for name, (shape, npdt) in input_specs(cfg).items():
        ins[name] = nc.dram_tensor(name, list(shape), mybir.dt.from_np(np.dtype(npdt)),
                                   kind="ExternalInput").ap()
    outs = {"xnew": nc.dram_tensor("xnew", [cfg["Sr"], D], mybir.dt.bfloat16,
                                   kind="ExternalOutput").ap()}
    with tile.TileContext(nc) as tc:
        build(tc, ins, outs, cfg)
    nc.compile()
    _CACHE[key] = nc
    return nc


def _enable_jax_persistent_cache():
    import jax
    try:
        jax.config.update("jax_compilation_cache_dir", JAX_CACHE_DIR)
        jax.config.update("jax_persistent_cache_min_compile_time_secs", 0.0)
        jax.config.update("jax_persistent_cache_min_entry_size_bytes", 0)
    except Exception:
        pass


def kernel(**inputs):
    from concourse.bass_utils import run_bass_kernel_spmd

    _enable_jax_persistent_cache()
    n_cores = 8
    cfg, in_maps = _host_prep(
        inputs["x"], inputs["edge_index"], inputs["edge_attr"],
        inputs["congestion"], inputs["mW1"], inputs["mb1"], inputs["mW2"],
        inputs["mb2"], inputs["uW1"], inputs["ub1"], inputs["uW2"],
        inputs["ub2"], n_cores)
    nc = _compiled(cfg)
    import time as _time
    _t0 = _time.time()
    def _reset_jax():
        # the terminal-side device sometimes wedges (NRT_EXEC_UNIT_
        # UNRECOVERABLE / mesh desynced), poisoning the PJRT client;
        # drop every jit bound to it and force a fresh client.
        try:
            import jax
            import jax.extend.backend
            _RUN_CACHE.clear()
            jax.clear_caches()
            jax.extend.backend.clear_backends()
        except Exception:
            pass

    want_trace = bool(os.environ.get("KERNEL_TRACE"))
    if want_trace and not _install_ntff_hook():
        want_trace = False

    res = None
    last_exc = None
    for i, (delay, use_lib) in enumerate(
            ((0, want_trace), (5, want_trace), (15, True), (30, False))):
        if delay:
            _time.sleep(delay)
        if i:
            _reset_jax()
        try:
            if use_lib:
                res = run_bass_kernel_spmd(nc, in_maps,
                                           core_ids=list(range(n_cores)),
                                           trace=want_trace)
            else:
                res = _run_pjrt(nc, in_maps, n_cores)
            break
        except Exception as e:
            last_exc = e
    if res is None:
        raise last_exc
    kernel.last_results = res
    kernel.last_exec_wall_s = _time.time() - _t0
    out = np.concatenate([r["xnew"] for r in res.results], axis=0)
    return np.ascontiguousarray(out[:cfg["N"]]).astype(np.float32)


# revision 17
# speedup vs baseline: 1.1434x; 1.1434x over previous
"""Trainium2 Bass kernel for CongestionAwareMP (GNN message passing).

Math (reference):
    msg_in = [x[src], x[dst], edge_attr, cong[src]]          # [E, 289]
    h      = relu(msg_in @ mW1 + mb1)                        # [E, 256]
    msgs   = h @ mW2 + mb2                                   # [E, 128]
    agg    = segment_sum(msgs, dst, N)                       # [N, 128]
    h2     = relu([x, agg] @ uW1 + ub1)                      # [N, 256]
    out    = h2 @ uW2 + ub2                                  # [N, 128]

Kernel decomposition (linear-algebra identities, exact up to bf16 rounding):
  * mW1 splits by input block:  h = relu(A[src] + B[dst] + ea @ W1e)
      A = x @ mW1[:128] + cong * mW1[288] + mb1   (per-node table)
      B = x @ mW1[128:256]                        (per-node table)
  * segment_sum commutes with the linear mW2 map:
      agg = segment_sum(h) @ mW2 + deg * mb2
  * mW2 folds into the update MLP (host-side weight product):
      h2 = relu(x @ uW1a + hsum @ V + deg * c + ub1),  V = mW2 @ uW1b

Distribution: edges sharded by dst node range (node-parallel).  Each core
computes the A-table rows for its own node shard, then the tables are
exchanged with one on-device AllGather (6.4 MB/core).  A[src] rows are
fetched per edge with a gpsimd DMA gather (the per-index descriptor rate
of ~8 ns/row is the kernel's floor); B[dst] is selected on the PE array
via an exact one-hot built from a partition-broadcast of the dst ids —
no second gather.  Aggregation uses the same one-hot transposed.
"""

import math
import os
import sys

sys.path.insert(0, "/opt/trn_rl_repo")

import ml_dtypes
import numpy as np

BF16 = ml_dtypes.bfloat16

P = 128          # partitions
WIN = 256        # dst-window (nodes) for aggregation
NG = 4           # src-range groups (int16 gather index limit)
D = 128          # node feature dim
ED = 32          # edge feature dim
HID = 256        # hidden dim

JAX_CACHE_DIR = "/tmp/bass_jax_cache"


def _cfg(n_nodes, n_edges, n_cores):
    Sr = int(math.ceil(n_nodes / (n_cores * WIN))) * WIN  # nodes per core
    Npad = Sr * n_cores              # global padded node space
    GRP = Npad // NG                 # == 2 * Sr when NG == n_cores // 2
    return dict(N=n_nodes, E=n_edges, NC=n_cores, Sr=Sr, Npad=Npad,
                Wr=Sr // WIN, GRP=GRP)


def _host_prep(x, edge_index, edge_attr, congestion,
               mW1, mb1, mW2, mb2, uW1, ub1, uW2, ub2, n_cores):
    cfg = _cfg(x.shape[0], edge_index.shape[1], n_cores)
    N, E, NC, Sr, Npad, Wr, GRP = (cfg[k] for k in
                                   ("N", "E", "NC", "Sr", "Npad", "Wr", "GRP"))

    x = np.asarray(x, np.float32)
    ea = np.asarray(edge_attr, np.float32)
    cong = np.asarray(congestion, np.float32)
    src = np.asarray(edge_index[0]).astype(np.int32, copy=False)
    dst = np.asarray(edge_index[1]).astype(np.int32, copy=False)
    mW1 = np.asarray(mW1, np.float32); mb1 = np.asarray(mb1, np.float32)
    mW2 = np.asarray(mW2, np.float32); mb2 = np.asarray(mb2, np.float32)
    uW1 = np.asarray(uW1, np.float32); ub1 = np.asarray(ub1, np.float32)
    uW2 = np.asarray(uW2, np.float32); ub2 = np.asarray(ub2, np.float32)

    # ---- global ordering by (dst-window, src-group) ----
    # single combined sort: key * E + edge_id  (stable by construction)
    key = (dst // WIN) * NG + src // GRP
    comb = key.astype(np.int64) * E + np.arange(E, dtype=np.int64)
    comb.sort()
    keys = (comb // E).astype(np.int32)
    order = (comb % E).astype(np.int32)

    nbuck = NC * Wr * NG
    gcnt = np.bincount(keys, minlength=nbuck)
    T_G = max(1, int(math.ceil(gcnt.max() / P)))
    T_W = NG * T_G
    Tt = Wr * T_W                    # tiles per core (real windows only)
    cfg.update(T_G=T_G, T_W=T_W, Tt=Tt)

    kstart = np.zeros(nbuck, np.int64)
    np.cumsum(gcnt[:-1], out=kstart[1:])
    rank = np.arange(E, dtype=np.int64) - kstart[keys]
    assert rank.max(initial=0) < T_G * P
    core = keys // (Wr * NG)
    klocal = keys - core * (Wr * NG)
    gslot = core * (Tt * P) + klocal * (T_G * P) + rank
    nslots_g = NC * Tt * P

    src_s = src[order]
    dst_s = dst[order]
    srcl_g = np.zeros(nslots_g, np.int16)
    srcl_g[gslot] = (src_s % GRP).astype(np.int16)
    dstl16 = (dst_s % WIN).astype(np.int16)
    dstf_g = np.full(nslots_g, -1.0, np.float32)
    dstf_g[gslot] = dstl16.astype(np.float32)      # 0..255 exact
    dstr_g = np.full(nslots_g, -1.0, BF16)
    dstr_g[gslot] = dstl16.astype(BF16)            # 0..255 exact in bf16
    eat_g = np.zeros((ED, nslots_g), BF16)
    eat_g[:, gslot] = ea.astype(BF16)[order].T

    deg_full = np.bincount(dst, minlength=Npad).astype(BF16)

    # ---- shared (replicated) small tensors ----
    w1ab = np.concatenate([mW1[0:D], mW1[D:2 * D]], axis=1).astype(BF16)  # [128, 512]
    w1cb1 = np.stack([mW1[2 * D + ED], mb1]).astype(BF16)                 # [2, 256]
    w1e = mW1[2 * D:2 * D + ED].astype(BF16)                              # [32, 256]
    uW1a = uW1[0:D].astype(BF16)
    uW1b = uW1[D:2 * D]
    V = (mW2 @ uW1b).astype(BF16)
    c2 = (mb2 @ uW1b)[None, :].astype(BF16)
    iota = np.broadcast_to(np.arange(WIN, dtype=np.float32), (P, WIN)).astype(BF16)
    iotacol = np.stack([np.arange(P, dtype=np.float32),
                        np.arange(P, dtype=np.float32) + P], axis=1)
    shared = dict(
        w1ab=w1ab, w1cb1=w1cb1, w1e=w1e, uw1a=uW1a,
        v0=V[0:P].copy(), v1=V[P:2 * P].copy(), c2=c2,
        uw2a=uW2[0:P].astype(BF16), uw2b=uW2[P:2 * P].astype(BF16),
        ub2=ub2[None, :].astype(BF16), ones1=np.ones((1, D), BF16),
        ub1c=np.stack([ub1[0:P], ub1[P:2 * P]], axis=1).astype(np.float32),
        iota=np.ascontiguousarray(iota),
        iotacol=np.ascontiguousarray(iotacol),
    )

    # x^T in bf16 once, then per-core column slices
    xT = np.ascontiguousarray(x.astype(BF16).T)   # [128, N]

    in_maps = []
    for c in range(NC):
        r0, r1 = c * Sr, min((c + 1) * Sr, N)
        xsh = np.zeros((P, Sr), BF16)
        xsh[:, :r1 - r0] = xT[:, r0:r1]
        cosh = np.ones((2, Sr), BF16)
        cosh[0] = 0.0
        cosh[0, :r1 - r0] = cong[r0:r1].astype(BF16)

        sl = slice(c * Tt * P, (c + 1) * Tt * P)
        m = dict(shared)
        m["xsh"] = xsh
        m["cosh"] = cosh
        Wr_, T_W_ = Wr, Tt // Wr
        m["srclc"] = np.ascontiguousarray(
            srcl_g[sl].reshape(Wr_, T_W_ * 8, 16).transpose(0, 2, 1))     # [Wr, 16, T_W*8]
        m["dstfc"] = np.ascontiguousarray(
            dstf_g[sl].reshape(Wr_, T_W_, P).transpose(0, 2, 1))          # [Wr, 128, T_W] f32
        m["dstr"] = dstr_g[None, sl]                                      # [1, Tt*128]
        m["eat"] = eat_g[:, sl]                                           # [32, Tt*128]
        m["deg"] = deg_full[None, c * Sr:(c + 1) * Sr]                    # [1, Sr]
        in_maps.append(m)

    return cfg, in_maps


def input_specs(cfg):
    Sr, Tt = cfg["Sr"], cfg["Tt"]
    return {
        "xsh": ((P, Sr), BF16), "cosh": ((2, Sr), BF16),
        "w1ab": ((P, 2 * HID), BF16), "w1cb1": ((2, HID), BF16),
        "w1e": ((ED, HID), BF16), "uw1a": ((P, HID), BF16),
        "v0": ((P, HID), BF16), "v1": ((P, HID), BF16),
        "c2": ((1, HID), BF16), "uw2a": ((P, D), BF16), "uw2b": ((P, D), BF16),
        "ub2": ((1, D), BF16), "ones1": ((1, D), BF16),
        "ub1c": ((P, 2), np.float32), "iota": ((P, WIN), BF16),
        "iotacol": ((P, 2), np.float32),
        "srclc": ((Sr // WIN, 16, (Tt // (Sr // WIN)) * 8), np.int16),
        "dstfc": ((Sr // WIN, P, Tt // (Sr // WIN)), np.float32),
        "dstr": ((1, Tt * P), BF16),
        "eat": ((ED, Tt * P), BF16),
        "deg": ((1, Sr), BF16),
    }


def build(tc, ins, outs, cfg):
    """Emit the Tile kernel.  ins/outs: dict name -> bass.AP (DRAM)."""
    from contextlib import ExitStack

    import concourse.mybir as mybir
    from concourse.tile_rust import add_dep_helper

    nc = tc.nc
    dt = mybir.dt
    AF = mybir.ActivationFunctionType
    ALU = mybir.AluOpType
    NC, Sr, Npad, Wr, T_W, T_G, GRP, Tt = (
        cfg[k] for k in ("NC", "Sr", "Npad", "Wr", "T_W", "T_G", "GRP", "Tt"))

    with ExitStack() as ctx:
        wp = ctx.enter_context(tc.tile_pool(name="wts", bufs=1))

        def load_w(name, shape, dty=dt.bfloat16):
            t = wp.tile(list(shape), dty, name=f"w_{name}")
            nc.sync.dma_start(out=t[:], in_=ins[name][:])
            return t

        w1ab = load_w("w1ab", (P, 2 * HID))
        w1cb1 = load_w("w1cb1", (2, HID))
        w1e = load_w("w1e", (ED, HID))
        uw1a = load_w("uw1a", (P, HID))
        v0 = load_w("v0", (P, HID))
        v1 = load_w("v1", (P, HID))
        c2 = load_w("c2", (1, HID))
        uw2a = load_w("uw2a", (P, D))
        uw2b = load_w("uw2b", (P, D))
        ub2 = load_w("ub2", (1, D))
        ones1 = load_w("ones1", (1, D))
        ub1c = load_w("ub1c", (P, 2), dt.float32)
        iota = load_w("iota", (P, WIN))
        iotacol = load_w("iotacol", (P, 2), dt.float32)
        deg = load_w("deg", (1, Sr))

        # A table is exchanged via AllGather; B stays core-local.
        agin = nc.dram_tensor("agin", [Sr, HID], dt.bfloat16, kind="Internal")
        agout = nc.dram_tensor("agout", [NC, Sr, HID], dt.bfloat16,
                               kind="Internal", addr_space="Shared")
        Bt = nc.dram_tensor("Bt", [Sr, HID], dt.bfloat16, kind="Internal")
        BtA = Bt.ap()

        # ---------- broadcast gather indices to 8 gpsimd cores ----------
        # [Wr, P, T_W*8]: each window's replicated index block is contiguous,
        # so the per-window SBUF load is one big descriptor, not 128 strided.
        sidx = nc.dram_tensor("sidx", [Wr, P, T_W * 8], dt.int16, kind="Internal")
        idx_writes = []
        for k in range(8):
            idx_writes.append(nc.sync.dma_start(
                out=sidx.ap()[:, 16 * k:16 * (k + 1), :], in_=ins["srclc"][:]))
        jidx = nc.engines[mybir.EngineType.SP].nop(nofuse=True, hint="idx_ready")
        for wi in idx_writes:
            add_dep_helper(jidx.ins, wi.ins, reason="idx bcast write")

        # ---------- per-node tables: A (own shard) + B, then AllGather A ----
        a_writes = []
        b_writes = {}          # chunk index -> join instruction
        with tc.tile_pool(name="pp", bufs=3) as pp, \
             tc.tile_pool(name="pps", bufs=3, space="PSUM") as pps:
            chunks = [(o, 512) for o in range(0, Sr - 511, 512)]
            if Sr % 512:
                chunks.append((Sr - Sr % 512, Sr % 512))

            for ci, (off, csz) in enumerate(chunks):
                xs = pp.tile([P, csz], dt.bfloat16, name="xs4")
                nc.sync.dma_start(out=xs[:], in_=ins["xsh"][:, off:off + csz])
                co4 = pp.tile([2, csz], dt.bfloat16, name="co4")
                nc.sync.dma_start(out=co4[:], in_=ins["cosh"][:, off:off + csz])
                cwr = []
                for k in range(csz // P):
                    ps = pps.tile([P, 2 * HID], dt.float32, name="ppsb")
                    nc.tensor.matmul(ps[:, 0:HID], lhsT=xs[:, k * P:(k + 1) * P],
                                     rhs=w1ab[:, 0:HID], start=True, stop=False)
                    nc.tensor.matmul(ps[:, 0:HID], lhsT=co4[:, k * P:(k + 1) * P],
                                     rhs=w1cb1[:], start=False, stop=True)
                    nc.tensor.matmul(ps[:, HID:2 * HID], lhsT=xs[:, k * P:(k + 1) * P],
                                     rhs=w1ab[:, HID:2 * HID], start=True, stop=True)
                    cb = pp.tile([P, 2 * HID], dt.bfloat16, name="cbuf")
                    nc.scalar.copy(out=cb[:, 0:HID], in_=ps[:, 0:HID])
                    nc.vector.tensor_copy(out=cb[:, HID:2 * HID], in_=ps[:, HID:2 * HID])
                    wa = nc.sync.dma_start(
                        out=agin.ap()[off + k * P:off + (k + 1) * P, :],
                        in_=cb[:, 0:HID])
                    a_writes.append(wa)
                    wb = nc.sync.dma_start(
                        out=BtA[off + k * P:off + (k + 1) * P, :],
                        in_=cb[:, HID:2 * HID])
                    cwr.append(wb)
                jb = nc.engines[mybir.EngineType.SP].nop(nofuse=True,
                                                         hint=f"B_{ci}")
                for wi in cwr:
                    add_dep_helper(jb.ins, wi.ins, reason="B chunk write")
                b_writes[ci] = jb

        cc = nc.gpsimd.collective_compute(
            "AllGather", ALU.bypass, [list(range(NC))],
            ins=[agin.ap()[:, :]], outs=[agout.ap()[:, :, :]])
        for wa in a_writes:
            add_dep_helper(cc.ins, wa.ins, reason="allgather after A build")

        def bjoin(w):
            # B rows for window w live in column chunk (w*WIN)//512
            return b_writes[min((w * WIN) // 512, len(b_writes) - 1)]

        # ---------- edge phase + fused update ----------
        with tc.tile_pool(name="ew", bufs=3) as ew, \
             tc.tile_pool(name="esg", bufs=3) as esg, \
             tc.tile_pool(name="esb", bufs=3) as esb, \
             tc.tile_pool(name="es", bufs=2) as es, \
             tc.tile_pool(name="eo", bufs=6) as eo, \
             tc.tile_pool(name="eps", bufs=2, space="PSUM") as eps, \
             tc.tile_pool(name="aps", bufs=2, space="PSUM") as aps, \
             tc.tile_pool(name="up", bufs=2) as up, \
             tc.tile_pool(name="ups", bufs=1, space="PSUM") as ups, \
             tc.tile_pool(name="ops", bufs=1, space="PSUM") as ops:
            pending_update = [None]

            def flush_update():
                if pending_update[0] is not None:
                    pending_update[0]()
                    pending_update[0] = None

            for w in range(Wr):
                t0 = w * T_W
                srcl_w = ew.tile([P, T_W * 8], dt.int16, name="srcl_w")
                ld = nc.sync.dma_start(out=srcl_w[:], in_=sidx.ap()[w, :, :])
                add_dep_helper(ld.ins, jidx.ins, reason="idx after bcast")
                dstf_w = ew.tile([P, T_W], dt.float32, name="dstf_w")
                nc.sync.dma_start(out=dstf_w[:], in_=ins["dstfc"][w, :, :])
                xtw = ew.tile([P, WIN], dt.bfloat16, name="xtw")
                nc.sync.dma_start(out=xtw[:], in_=ins["xsh"][:, w * WIN:(w + 1) * WIN])
                dstbc = esb.tile([P, T_W * P], dt.bfloat16, name="dstbc")
                nq = T_W * P // 4
                for q in range(4):
                    nc.sync.dma_start(
                        out=dstbc[:, q * nq:(q + 1) * nq],
                        in_=ins["dstr"][0:1, t0 * P + q * nq:
                                        t0 * P + (q + 1) * nq].partition_broadcast(P))
                eat_w = es.tile([ED, T_W * P], dt.bfloat16, name="eat_w")
                nc.sync.dma_start(out=eat_w[:], in_=ins["eat"][:, t0 * P:(t0 + T_W) * P])
                bw = ew.tile([P, 2 * HID], dt.bfloat16, name="bw")
                for g in range(2):
                    lb = nc.sync.dma_start(
                        out=bw[:, g * HID:(g + 1) * HID],
                        in_=BtA[w * WIN + g * P:w * WIN + (g + 1) * P, :])
                    add_dep_helper(lb.ins, bjoin(w).ins, reason="Bw after B build")

                GA = esg.tile([P, T_W * HID], dt.bfloat16, name="GA")
                for g in range(NG):
                    ga = nc.gpsimd.dma_gather(
                        GA[:, g * T_G * HID:(g + 1) * T_G * HID].rearrange(
                            "p (t e) -> p t e", e=HID),
                        agout.ap()[2 * g:2 * g + 2, :, :].rearrange(
                            "c s h -> (c s) h"),
                        srcl_w[:, g * T_G * 8:(g + 1) * T_G * 8],
                        T_G * P, T_G * P, HID,
                        single_packet=False, queue_num=g)
                    add_dep_helper(ga.ins, cc.ins, reason="gatherA after AG")

                # hoisted one-hot builds for the whole window (no chain deps).
                # Split across engines: vector via is_equal, scalar via the
                # exact integer identity  onehot = relu(1 - |iota - dst|).
                # fp8 output: 0/1 are exact, and it feeds the DoubleRow
                # aggregation matmul directly.
                ohb = es.tile([P, T_W * WIN], dt.float8e4, name="ohb")
                for t in range(T_W):
                    sl = ohb[:, t * WIN:(t + 1) * WIN]
                    if t % 2 == 0:
                        nc.vector.tensor_scalar(sl, iota[:],
                                                dstf_w[:, t:t + 1], None,
                                                op0=ALU.is_equal)
                    else:
                        nc.scalar.activation(sl, iota[:], AF.Abs,
                                             bias=dstf_w[:, t:t + 1], scale=-1.0)
                        nc.scalar.activation(sl, sl, AF.Relu, bias=1.0, scale=-1.0)
                hbuf = es.tile([P, T_W * HID], dt.float8e4, name="hbuf")

                hs0 = aps.tile([P, WIN], dt.float32, name="hsum0")
                hs1 = aps.tile([P, WIN], dt.float32, name="hsum1")

                # software-pipelined: chunk i's select matmuls are issued
                # before chunk i-1's aggregation matmuls so the in-order PE
                # queue never waits on the vector/scalar add+relu of i-1.
                NCH = T_W // 2
                ps2s = [None] * NCH
                ohTs = [None] * NCH
                for i in range(NCH + 2):
                    if i < NCH:
                        if i % 2 == 0:
                            # one-hot^T batched over 4 tiles (two chunks)
                            nb = min(4 * P, (T_W - 2 * i) * P)
                            ohT0 = eo.tile([P, 4 * P], dt.bfloat16, name="ohT0")
                            nc.vector.tensor_scalar(
                                ohT0[:, 0:nb],
                                dstbc[:, i * 2 * P:i * 2 * P + nb],
                                iotacol[:, 0:1], None, op0=ALU.is_equal)
                            ohT1 = eo.tile([P, 4 * P], dt.bfloat16, name="ohT1")
                            nc.vector.tensor_scalar(
                                ohT1[:, 0:nb],
                                dstbc[:, i * 2 * P:i * 2 * P + nb],
                                iotacol[:, 1:2], None, op0=ALU.is_equal)
                            ohTs[i] = (ohT0, ohT1, 0)
                            if i + 1 < NCH:
                                ohTs[i + 1] = (ohT0, ohT1, 2 * P)
                        ohT0, ohT1, obase = ohTs[i]
                        ps2 = eps.tile([P, 2 * HID], dt.float32, name="ps2")
                        ps2s[i] = ps2
                        for j in range(2):
                            t = 2 * i + j
                            slp = ps2[:, j * HID:(j + 1) * HID]
                            nc.tensor.matmul(slp, lhsT=eat_w[:, t * P:(t + 1) * P],
                                             rhs=w1e[:], start=True, stop=False)
                            nc.tensor.matmul(slp,
                                             lhsT=ohT0[:, obase + j * P:obase + (j + 1) * P],
                                             rhs=bw[:, 0:HID], start=False, stop=False)
                            nc.tensor.matmul(slp,
                                             lhsT=ohT1[:, obase + j * P:obase + (j + 1) * P],
                                             rhs=bw[:, HID:2 * HID], start=False,
                                             stop=True)
                    if 1 <= i <= NCH:
                        c = i - 1
                        GA2 = GA[:, c * 2 * HID:(c + 1) * 2 * HID]
                        nc.vector.tensor_tensor(out=GA2, in0=GA2, in1=ps2s[c][:],
                                                op=ALU.add)
                        hb2 = hbuf[:, c * 2 * HID:(c + 1) * 2 * HID]
                        nc.scalar.activation(hb2, GA2, AF.Relu)
                    if i >= 2:
                        c = i - 2
                        # DoubleRow aggregation: both tiles of the chunk in
                        # one matmul per hid-half (K = 2x128 edge slots).
                        hb2 = hbuf[:, c * 2 * HID:(c + 1) * 2 * HID]
                        oh_r = ohb[:, 2 * c * WIN:(2 * c + 2) * WIN].rearrange(
                            "p (k n) -> p k n", k=2)
                        hb_r = hb2.rearrange("p (k n) -> p k n", k=2)
                        first, last = (c == 0), (c == NCH - 1)
                        nc.tensor.matmul(hs0[:], lhsT=hb_r[:, :, 0:P], rhs=oh_r,
                                         start=first, stop=last,
                                         perf_mode=mybir.MatmulPerfMode.DoubleRow)
                        nc.tensor.matmul(hs1[:], lhsT=hb_r[:, :, P:2 * P], rhs=oh_r,
                                         start=first, stop=last,
                                         perf_mode=mybir.MatmulPerfMode.DoubleRow)

                # ----- update MLP for this window's nodes -----
                # Deferred into the next window's chunk loop so the PE queue
                # is never drained at a window boundary.
                def make_update(w, hs0, hs1, xtw):
                    def emit():
                        hsb = up.tile([P, 2 * HID], dt.bfloat16, name="hsb")
                        nc.vector.tensor_copy(out=hsb[:, 0:HID], in_=hs0[:])
                        nc.vector.tensor_copy(out=hsb[:, HID:2 * HID], in_=hs1[:])
                        psu = ups.tile([P, 2 * HID], dt.float32, name="psu")
                        for j in range(2):
                            slu = psu[:, j * HID:(j + 1) * HID]
                            nc.tensor.matmul(slu, lhsT=uw1a[:, j * P:(j + 1) * P],
                                             rhs=xtw[:], start=True, stop=False)
                            nc.tensor.matmul(slu, lhsT=v0[:, j * P:(j + 1) * P],
                                             rhs=hsb[:, 0:HID], start=False, stop=False)
                            nc.tensor.matmul(slu, lhsT=v1[:, j * P:(j + 1) * P],
                                             rhs=hsb[:, HID:2 * HID], start=False,
                                             stop=False)
                            nc.tensor.matmul(slu, lhsT=c2[:, j * P:(j + 1) * P],
                                             rhs=deg[:, w * WIN:(w + 1) * WIN],
                                             start=False, stop=True)
                        h2 = up.tile([P, 2 * HID], dt.bfloat16, name="h2")
                        for j in range(2):
                            nc.scalar.activation(h2[:, j * HID:(j + 1) * HID],
                                                 psu[:, j * HID:(j + 1) * HID],
                                                 AF.Relu, bias=ub1c[:, j:j + 1])
                        pso = ops.tile([P, WIN], dt.float32, name="pso")
                        for sx in range(2):
                            slo = pso[:, sx * D:(sx + 1) * D]
                            nc.tensor.matmul(slo, lhsT=h2[:, sx * P:sx * P + P],
                                             rhs=uw2a[:], start=True, stop=False)
                            nc.tensor.matmul(slo,
                                             lhsT=h2[:, HID + sx * P:HID + sx * P + P],
                                             rhs=uw2b[:], start=False, stop=False)
                            nc.tensor.matmul(slo, lhsT=ones1[:], rhs=ub2[:],
                                             start=False, stop=True)
                            osb = up.tile([P, D], dt.bfloat16, name="osb")
                            nc.scalar.copy(out=osb[:], in_=slo)
                            nc.sync.dma_start(
                                out=outs["xnew"][w * WIN + sx * P:
                                                 w * WIN + (sx + 1) * P, :],
                                in_=osb[:])
                    return emit

                make_update(w, hs0, hs1, xtw)()
            flush_update()


def _install_ntff_hook():
    """Make ``antenv.axon_hooks`` importable so bass_utils' axon trace path
    finds the NTFF profile hook (the boot skips registration when the
    image's antenv package lacks the module).  Returns True when the hook
    is available, False otherwise (tracing then degrades gracefully)."""
    import types

    try:
        from antenv.axon_hooks import get_axon_ntff_profile_hook
        return get_axon_ntff_profile_hook() is not None
    except ImportError:
        pass
    try:
        from trn_agent_boot.trn_boot import _ntff_profile_via_ctypes
        hook = _ntff_profile_via_ctypes("/opt/axon/libaxon_pjrt.so")
    except Exception:
        return False
    if hook is None:
        return False
    mod = types.ModuleType("antenv.axon_hooks")
    mod._hook = hook
    mod.get_axon_ntff_profile_hook = lambda: mod._hook
    mod.set_axon_ntff_profile_hook = lambda h: setattr(mod, "_hook", h)
    sys.modules["antenv.axon_hooks"] = mod
    return True


_RUN_CACHE = {}


def _run_pjrt(nc, in_maps, n_cores):
    """Mirror of bass2jax.run_bass_via_pjrt with two transfer
    optimizations for the ~70 MB/s axon host link: the donated
    ExternalOutput zero buffers are created on-device (saves shipping
    26 MB of zeros), and host->device uploads run on a small thread
    pool (the tunnel gains ~10% with concurrent streams)."""
    from concurrent.futures import ThreadPoolExecutor

    import jax
    import jax.numpy as jnp
    from jax.sharding import Mesh, NamedSharding, PartitionSpec
    from jax.experimental.shard_map import shard_map

    import concourse.mybir as mybir
    from concourse.bass2jax import (_bass_exec_p, install_neuronx_cc_hook,
                                    partition_id_tensor)

    key = id(nc)
    if key not in _RUN_CACHE:
        install_neuronx_cc_hook()
        partition_name = (nc.partition_id_tensor.name
                          if nc.partition_id_tensor else None)
        in_names, out_names, out_avals = [], [], []
        for alloc in nc.m.functions[0].allocations:
            if not isinstance(alloc, mybir.MemoryLocationSet):
                continue
            name = alloc.memorylocations[0].name
            if alloc.kind == "ExternalInput":
                if name != partition_name:
                    in_names.append(name)
            elif alloc.kind == "ExternalOutput":
                out_names.append(name)
                out_avals.append(jax.core.ShapedArray(
                    tuple(alloc.tensor_shape), mybir.dt.np(alloc.dtype)))
        n_params = len(in_names)
        n_outs = len(out_avals)
        in_names = in_names + out_names
        if partition_name is not None:
            in_names.append(partition_name)

        def _body(*args):
            operands = list(args)
            if partition_name is not None:
                operands.append(partition_id_tensor())
            return tuple(_bass_exec_p.bind(
                *operands, out_avals=tuple(out_avals),
                in_names=tuple(in_names), out_names=tuple(out_names),
                lowering_input_output_aliases=(),
                sim_require_finite=True, sim_require_nnan=True, nc=nc))

        devices = jax.devices()[:n_cores]
        assert len(devices) == n_cores
        mesh = Mesh(np.asarray(devices), ("core",))
        spec = PartitionSpec("core")
        sh = NamedSharding(mesh, spec)
        sharded = jax.jit(
            shard_map(_body, mesh=mesh, in_specs=(spec,) * (n_params + n_outs),
                      out_specs=(spec,) * n_outs),
            donate_argnums=tuple(range(n_params, n_params + n_outs)),
            keep_unused=True)
        zeros_fn = jax.jit(
            lambda: tuple(jnp.zeros((n_cores * a.shape[0], *a.shape[1:]),
                                    a.dtype) for a in out_avals),
            out_shardings=(sh,) * n_outs)
        _RUN_CACHE[key] = (in_names, out_names, out_avals, n_params, n_outs,
                           sh, sharded, zeros_fn)
    (in_names, out_names, out_avals, n_params, n_outs,
     sh, sharded, zeros_fn) = _RUN_CACHE[key]

    concat_in = [
        np.concatenate([np.asarray(m[name]) for m in in_maps], axis=0)
        for name in in_names[:n_params]]
    dev_zeros = zeros_fn()

    # byte-balanced upload groups over a few concurrent tunnel streams
    order = sorted(range(n_params), key=lambda i: -concat_in[i].nbytes)
    groups = [[] for _ in range(4)]
    sizes = [0] * 4
    for i in order:
        g = sizes.index(min(sizes))
        groups[g].append(i)
        sizes[g] += concat_in[i].nbytes
    dev_in = [None] * n_params

    def _upload(group):
        for i in group:
            dev_in[i] = jax.device_put(concat_in[i], sh)
        for i in group:
            dev_in[i].block_until_ready()

    with ThreadPoolExecutor(4) as ex:
        list(ex.map(_upload, groups))

    out_arrs = sharded(*dev_in, *dev_zeros)
    for o in out_arrs:
        o.block_until_ready()
    results = [
        {name: np.asarray(out_arrs[i]).reshape(n_cores, *out_avals[i].shape)[c]
         for i, name in enumerate(out_names)}
        for c in range(n_cores)]

    class _Res:
        pass

    res = _Res()
    res.results = results
    res.exec_time_ns = None
    return res


_CACHE = {}


def _compiled(cfg):
    key = ("v2", cfg["N"], cfg["E"], cfg["NC"], cfg["T_W"])
    if key in _CACHE:
        return _CACHE[key]
    import concourse.mybir as mybir
    import concourse.tile as tile
    from concourse import bacc

    nc = bacc.Bacc("TRN2", target_bir_lowering=False, debug=False,
                   enable_asserts=False, num_devices=cfg["NC"],
                   num_swdge_queues=4)
    ins = {}
    for name, (shape, npdt) in input_specs(cfg).items():
        ins[name] = nc.dram_tensor(name, list(shape), mybir.dt.from_np(np.dtype(npdt)),
                                   kind="ExternalInput").ap()
    outs = {"xnew": nc.dram_tensor("xnew", [cfg["Sr"], D], mybir.dt.bfloat16,
                                   kind="ExternalOutput").ap()}
    with tile.TileContext(nc) as tc:
        build(tc, ins, outs, cfg)
    nc.compile()
    _CACHE[key] = nc
    return nc


def _enable_jax_persistent_cache():
    import jax
    try:
        jax.config.update("jax_compilation_cache_dir", JAX_CACHE_DIR)
        jax.config.update("jax_persistent_cache_min_compile_time_secs", 0.0)
        jax.config.update("jax_persistent_cache_min_entry_size_bytes", 0)
    except Exception:
        pass


def kernel(**inputs):
    from concourse.bass_utils import run_bass_kernel_spmd

    _enable_jax_persistent_cache()
    n_cores = 8
    cfg, in_maps = _host_prep(
        inputs["x"], inputs["edge_index"], inputs["edge_attr"],
        inputs["congestion"], inputs["mW1"], inputs["mb1"], inputs["mW2"],
        inputs["mb2"], inputs["uW1"], inputs["ub1"], inputs["uW2"],
        inputs["ub2"], n_cores)
    nc = _compiled(cfg)
    import time as _time
    _t0 = _time.time()
    def _reset_jax():
        # the terminal-side device sometimes wedges (NRT_EXEC_UNIT_
        # UNRECOVERABLE / mesh desynced), poisoning the PJRT client;
        # drop every jit bound to it and force a fresh client.
        try:
            import jax
            import jax.extend.backend
            _RUN_CACHE.clear()
            jax.clear_caches()
            jax.extend.backend.clear_backends()
        except Exception:
            pass

    want_trace = bool(os.environ.get("KERNEL_TRACE"))
    if want_trace and not _install_ntff_hook():
        want_trace = False

    res = None
    last_exc = None
    for i, (delay, use_lib) in enumerate(
            ((0, want_trace), (5, want_trace), (15, True), (30, False))):
        if delay:
            _time.sleep(delay)
        if i:
            _reset_jax()
        try:
            if use_lib:
                res = run_bass_kernel_spmd(nc, in_maps,
                                           core_ids=list(range(n_cores)),
                                           trace=want_trace)
            else:
                res = _run_pjrt(nc, in_maps, n_cores)
            break
        except Exception as e:
            last_exc = e
    if res is None:
        raise last_exc
    kernel.last_results = res
    kernel.last_exec_wall_s = _time.time() - _t0
    out = np.concatenate([r["xnew"] for r in res.results], axis=0)
    return np.ascontiguousarray(out[:cfg["N"]]).astype(np.float32)
